# revision 8
# baseline (speedup 1.0000x reference)
"""Trainium2 Bass kernel for nn_DeepModel_70703751626759 (deep-BSDE forward sim).

v3: two software-pipelined sample streams (column halves) so TensorE
never idles (HAM stays warm at 2.4 GHz); dw broadcast moved off the PE
onto DMA (host-replicated rows) so SCA/SCB are bf16 2x-mode DVE ops;
rotated PE tile placement (H1 at p+-1, H2 at p+2, SGR rhs at p+2) to
spread matmuls across the 4x4 sub-array grid; elementwise work split
across ScalarE (state copy + 2 tanh) and VectorE (biased ZU copy,
SCA/SCB, loss accum).

Data-parallel over 8 NeuronCores: 32768 samples -> 4096/core -> 8 column
blocks of 512 samples; block j = p + 4q at partitions [32p, 32p+32),
free cols [512q, 512q+512). Slot content: state = [X(16); Y(16)],
ZU = [u(8); Zv(16); dH(8)]. t folded into layer-1 biases; losses
reduced on-device via accum_out.
"""

import sys
import os
import numpy as np

if "/opt/trn_rl_repo" not in sys.path:
    sys.path.insert(0, "/opt/trn_rl_repo")

N = 16
M = 8
T = 50
DT = 0.01
GAMMA = 0.1
SIGMA = 0.2
TAU = 0.5
H = 10
BATCH = 32768
NCORES = 8
CB = BATCH // NCORES
BK = 512

F32 = np.float32
try:
    import ml_dtypes
    BF16 = ml_dtypes.bfloat16
except ImportError:          # pragma: no cover
    BF16 = np.float32

# bf16 consts image (ckb) column offsets
K_W1 = 0
K_W2 = 32
K_W3 = 64
K_WZY = 96
K_WDXY = 128
K_WDZU = 160
K_WDSA = 192
K_WDSB = 224
K_WE = 256
K_WY1 = 288
K_WY2 = 320
K_WY3 = 352
K_SGR = 384
K_BY3V = 416
K_ONES512 = 448
K_COLS = 1472

# fp32 consts image (ck) column offsets
C_WYP = 0
C_B1 = 32          # 64 cols (t = 0..T-1)
C_B2 = 96
C_B3 = 97
C_BY1 = 98
C_BY2 = 99
C_GB = 100         # 51 cols: t*gamma*dt on X rows (t = 0..T)
C_COLS = 160


def _ct(t):
    w = 1.0 if (t == 0 or t == T - 1) else 2.0
    return 0.5 * DT * w * TAU * TAU


def _b_of(p, q):
    return (p + (1 if q == 0 else 3)) % 4


def _weight_blocks(inp):
    A = np.asarray(inp["A"], F32)
    Bm = np.asarray(inp["Bmat"], F32)
    C = np.asarray(inp["Cmat"], F32)
    D = np.asarray(inp["Dmat"], F32)
    ZW1 = np.asarray(inp["Z_W1"], F32)
    ZW2 = np.asarray(inp["Z_W2"], F32)
    ZW3 = np.asarray(inp["Z_W3"], F32)
    PW1 = np.asarray(inp["phi_W1"], F32)
    PW2 = np.asarray(inp["phi_W2"], F32)
    PW3 = np.asarray(inp["phi_W3"], F32)
    YW1 = np.asarray(inp["Y0_W1"], F32)
    YW2 = np.asarray(inp["Y0_W2"], F32)
    YW3 = np.asarray(inp["Y0_W3"], F32)
    I16 = np.eye(16, dtype=F32)

    def blk():
        return np.zeros((32, 32), F32)

    W1 = blk()
    W1[0:16, 0:10] = ZW1[1:, :]
    W1[0:16, 10:20] = PW1[1:, :]
    W2 = blk()
    W2[0:10, 0:10] = ZW2
    W2[10:20, 10:20] = PW2
    W3 = blk()
    W3[10:20, 0:8] = PW3
    W3[0:10, 8:24] = ZW3
    W3[0:10, 24:32] = ZW3 @ D
    W3[10:20, 24:32] = PW3
    WZY = blk()
    WZY[16:32, 24:32] = Bm
    WDXY = blk()
    WDXY[0:16, 0:16] = DT * A.T
    WDXY[0:16, 16:32] = -DT * I16
    WDXY[16:32, 16:32] = -DT * A
    WDZU = blk()
    WDZU[0:8, 0:16] = DT * Bm.T
    WDZU[8:24, 16:32] = -DT * C
    WDSA = blk()
    WDSA[0:16, 0:16] = C.T
    WDSB = blk()
    WDSB[0:8, 0:16] = D.T
    WDSB[8:24, 16:32] = I16
    WE = blk()
    WE[0:16, 0:16] = -I16
    WE[16:32, 0:16] = I16
    WY1 = blk()
    WY1[0:16, 0:10] = YW1
    WY2 = blk()
    WY2[0:10, 0:10] = YW2
    WY3 = blk()
    WY3[0:10, 16:32] = YW3
    WYP = blk()
    WYP[0:16, 0:16] = I16
    SGR = np.zeros((1, 32), F32)      # K=1 lhsT row: sigma into X rows
    SGR[0, 0:16] = SIGMA
    GDT = np.zeros((1, 32), F32)      # K=1 lhsT row: gamma*dt into X rows
    GDT[0, 0:16] = GAMMA * DT
    BY3V = np.zeros((1, 32), F32)
    BY3V[0, 16:32] = np.asarray(inp["Y0_b3"], F32)
    return dict(W1=W1, W2=W2, W3=W3, WZY=WZY, WDXY=WDXY, WDZU=WDZU,
                WDSA=WDSA, WDSB=WDSB, WE=WE, WY1=WY1, WY2=WY2, WY3=WY3,
                WYP=WYP, SGR=SGR, GDT=GDT, BY3V=BY3V)


def pack_weights_bf16(inp):
    wb = _weight_blocks(inp)
    img = np.zeros((128, K_COLS), F32)
    reps = [(K_W1, "W1"), (K_W2, "W2"), (K_W3, "W3"), (K_WZY, "WZY"),
            (K_WDXY, "WDXY"), (K_WDZU, "WDZU"), (K_WDSA, "WDSA"),
            (K_WDSB, "WDSB"), (K_WE, "WE"), (K_WY1, "WY1"),
            (K_WY2, "WY2"), (K_WY3, "WY3")]
    for p in range(4):
        r = 32 * p
        for off, name in reps:
            img[r: r + 32, off: off + 32] = wb[name]
        img[r: r + 1, K_BY3V: K_BY3V + 32] = wb["BY3V"]
        img[r: r + 1, K_SGR: K_SGR + 32] = wb["SGR"]
    img[:, K_ONES512: K_ONES512 + 1024] = 1.0
    return img.astype(BF16)


def pack_weights_f32(inp):
    wb = _weight_blocks(inp)
    Zb1 = np.asarray(inp["Z_b1"], F32)
    Zb2 = np.asarray(inp["Z_b2"], F32)
    Zb3 = np.asarray(inp["Z_b3"], F32)
    Pb1 = np.asarray(inp["phi_b1"], F32)
    Pb2 = np.asarray(inp["phi_b2"], F32)
    Pb3 = np.asarray(inp["phi_b3"], F32)
    Yb1 = np.asarray(inp["Y0_b1"], F32)
    Yb2 = np.asarray(inp["Y0_b2"], F32)
    ZW1 = np.asarray(inp["Z_W1"], F32)
    PW1 = np.asarray(inp["phi_W1"], F32)
    D = np.asarray(inp["Dmat"], F32)
    img = np.zeros((128, C_COLS), F32)
    for t in range(T):
        tv = F32(t * DT)
        b = np.concatenate([Zb1 + tv * ZW1[0, :], Pb1 + tv * PW1[0, :]])
        for p in range(4):
            img[32 * p: 32 * p + 20, C_B1 + t] = b
    b2 = np.concatenate([Zb2, Pb2])
    b3 = np.concatenate([Pb3, Zb3, Pb3 + D.T @ Zb3])
    for p in range(4):
        r = 32 * p
        img[r: r + 32, C_WYP: C_WYP + 32] = wb["WYP"]
        img[r: r + 20, C_B2] = b2
        img[r: r + 32, C_B3] = b3
        img[r: r + 10, C_BY1] = Yb1
        img[r: r + 10, C_BY2] = Yb2
        for t in range(T + 1):
            img[r: r + 16, C_GB + t] = t * GAMMA * DT
    return img


def pack_x0(X0, core):
    out = np.zeros((128, 1024), F32)
    base = core * CB
    for p in range(4):
        for q in range(2):
            j = p + 4 * q
            out[32 * p: 32 * p + 16, 512 * q: 512 * (q + 1)] = \
                X0[base + BK * j: base + BK * (j + 1), :].T
    return np.ascontiguousarray(out)


def pack_pdw(dw, core):
    """[T*128, 1024] bf16: rows [128t+32p, +32) cols [512q, +512) =
    dw of block j = p+4q at step t, replicated across the 32 rows."""
    base = core * CB
    d = np.empty((T, 4, 2, BK), F32)
    for p in range(4):
        for q in range(2):
            j = p + 4 * q
            d[:, p, q, :] = dw[:, base + BK * j: base + BK * (j + 1), 0]
    # [T, 4, 2, BK] -> [T, 4p, 32rep, 2q*BK]
    out = np.broadcast_to(d[:, :, None, :, :], (T, 4, 32, 2, BK))
    out = out.reshape(T * 128, 1024)
    return np.ascontiguousarray(out.astype(BF16))


# ---------------------------------------------------------------------------
# device program
# ---------------------------------------------------------------------------

_BUILT = {}
STEPOPS = int(os.environ.get("K_STEPOPS", "10"))


def build(t_steps=T):
    if t_steps in _BUILT:
        return _BUILT[t_steps]
    from contextlib import ExitStack
    import concourse.tile as tile
    from concourse import bacc, mybir

    f32 = mybir.dt.float32
    bf16 = mybir.dt.bfloat16
    AF = mybir.ActivationFunctionType
    OP = mybir.AluOpType

    PF = 3                     # pdw prefetch depth (steps ahead)

    nc = bacc.Bacc("TRN2", target_bir_lowering=False, debug=False)
    pdw_d = nc.dram_tensor("pdw", [T * 128, 1024], bf16,
                           kind="ExternalInput").ap()
    x0p_d = nc.dram_tensor("x0p", [128, 1024], f32, kind="ExternalInput").ap()
    ckb_d = nc.dram_tensor("ckb", [128, K_COLS], bf16,
                           kind="ExternalInput").ap()
    ckf_d = nc.dram_tensor("ckf", [128, C_COLS], f32,
                           kind="ExternalInput").ap()
    lacc_d = nc.dram_tensor("out_lacc", [128, 128], f32,
                            kind="ExternalOutput").ap()
    eacc_d = nc.dram_tensor("out_eacc", [128, 8], f32,
                            kind="ExternalOutput").ap()

    def B32(tens, g):
        return tens[32 * g: 32 * g + 32, :]

    with tile.TileContext(nc) as tc, ExitStack() as ctx:
        sb = ctx.enter_context(tc.tile_pool(name="sb", bufs=1))
        dwpool = ctx.enter_context(tc.tile_pool(name="dwp", bufs=PF + 1))
        ps = ctx.enter_context(tc.tile_pool(name="ps", bufs=1, space="PSUM"))

        ckb = sb.tile([128, K_COLS], bf16, tag="ckb")
        ckf = sb.tile([128, C_COLS], f32, tag="ckf")
        nc.sync.dma_start(out=ckb[:, :], in_=ckb_d[:, :])
        nc.sync.dma_start(out=ckf[:, :], in_=ckf_d[:, :])
        X0SB = sb.tile([128, 1024], f32, tag="X0SB")
        nc.sync.dma_start(out=X0SB[:, :], in_=x0p_d[:, :])

        XY = [sb.tile([128, 512], bf16, tag=f"XY{q}", name=f"XY{q}") for q in range(2)]
        H1 = [sb.tile([128, 512], bf16, tag=f"H1{q}", name=f"H1{q}") for q in range(2)]
        H2 = [sb.tile([128, 512], bf16, tag=f"H2{q}", name=f"H2{q}") for q in range(2)]
        ZU = [sb.tile([128, 512], bf16, tag=f"ZU{q}", name=f"ZU{q}") for q in range(2)]
        SCA = [sb.tile([128, 512], bf16, tag=f"SCA{q}", name=f"SCA{q}") for q in range(2)]
        SCB = [sb.tile([128, 512], bf16, tag=f"SCB{q}", name=f"SCB{q}") for q in range(2)]
        SCR = [sb.tile([128, 512], bf16, tag=f"SCR{q}", name=f"SCR{q}") for q in range(2)]
        lacc = sb.tile([128, 128], f32, tag="lacc")
        eacc = sb.tile([128, 8], f32, tag="eacc")
        nc.vector.memset(lacc[:, :], 0.0)
        nc.vector.memset(eacc[:, :], 0.0)

        PST = [ps.tile([128, 512], f32, tag=f"pst{q}", name=f"pst{q}") for q in range(2)]
        DUM = ps.tile([128, 512], f32, tag="dum", name="DUM")
        _fill_g = [0]
        NDUM = int(os.environ.get("K_DUMMY", "1"))

        def filler(n=1):
            # Keep the PE array streaming through dependency stalls so the
            # HAM clock gate stays at 8/8. Runs on the otherwise-unused
            # (g, g+2) tile set into a scratch PSUM bank.
            for _ in range(n * NDUM):
                g = _fill_g[0] = (_fill_g[0] + 1) % 4
                d = (g + 2) % 4
                nc.tensor.matmul(
                    out=DUM[32 * d: 32 * d + 32, :],
                    lhsT=wkb(K_W1, g),
                    rhs=ckb[32 * g: 32 * g + 32, K_ONES512: K_ONES512 + 512],
                    start=True, stop=True,
                    tile_position=(32 * g, 32 * d),
                    skip_group_check=True)

        def wkb(off, g):
            return ckb[32 * g: 32 * g + 32, off: off + 32]

        def wkf(off, g):
            return ckf[32 * g: 32 * g + 32, off: off + 32]

        def bias(col):
            return ckf[:, col: col + 1]

        def mm(out_t, og, lhsT, rhs, start, stop, rg):
            nc.tensor.matmul(out=B32(out_t, og), lhsT=lhsT, rhs=rhs,
                             start=start, stop=stop,
                             tile_position=(32 * rg, 32 * og),
                             skip_group_check=True)

        # ---- warm-up burst: ~8.5us of back-to-back PE work ----
        filler(10)

        # ---- init: PST = WYP @ X0; PST[Y rows] += Y0-MLP(X0) ----
        for q in range(2):
            for p in range(4):
                mm(PST[q], p, wkf(C_WYP, p),
                   X0SB[32 * p: 32 * p + 32, 512 * q: 512 * q + 512],
                   True, False, p)
        ph_i = [ps.tile([128, 512], f32, tag=f"ph1{q}", name=f"ph1{q}") for q in range(2)]
        ph2_i = [ps.tile([128, 512], f32, tag=f"ph1{q}", name=f"ph2i{q}") for q in range(2)]
        for q in range(2):
            nc.scalar.activation(out=XY[q][:, :], in_=PST[q][:, :],
                                 func=AF.Copy)
        for q in range(2):
            for p in range(4):
                mm(ph_i[q], _b_of(p, q), wkb(K_WY1, p), B32(XY[q], p),
                   True, True, p)
        for q in range(2):
            nc.scalar.activation(out=H1[q][:, :], in_=ph_i[q][:, :],
                                 func=AF.Tanh, bias=bias(C_BY1))
        for q in range(2):
            for p in range(4):
                b = _b_of(p, q)
                mm(ph2_i[q], p, wkb(K_WY2, b), B32(H1[q], b), True, True, b)
        for q in range(2):
            nc.scalar.activation(out=H2[q][:, :], in_=ph2_i[q][:, :],
                                 func=AF.Tanh, bias=bias(C_BY2))
        for q in range(2):
            for p in range(4):
                mm(PST[q], p, wkb(K_WY3, p), B32(H2[q], p), False, False, p)
                mm(PST[q], p, ckb[32 * p: 32 * p + 1, K_BY3V: K_BY3V + 32],
                   ckb[32 * p: 32 * p + 1, K_ONES512: K_ONES512 + 512],
                   False, False, p)

        # ---- pdw prefetch for first PF steps ----
        pdw_tiles = {}

        def fetch_pdw(t):
            pt = dwpool.tile([128, 1024], bf16, tag="pdw", name="pdwt")
            nc.sync.dma_start(out=pt[:, :],
                              in_=pdw_d[128 * t: 128 * (t + 1), :])
            pdw_tiles[t] = pt

        for t in range(min(PF, t_steps)):
            fetch_pdw(t)

        # ---- steps ----
        for t in range(t_steps):
            if t + PF < t_steps:
                fetch_pdw(t + PF)
            pdwt = pdw_tiles.pop(t)
            last = (t == t_steps - 1)

            # state copy (+ t*gamma*dt on X rows): scalar / vector
            nc.scalar.activation(out=XY[0][:, :], in_=PST[0][:, :],
                                 func=AF.Identity, bias=bias(C_GB + t))
            nc.vector.tensor_scalar(XY[1][:, :], PST[1][:, :],
                                    bias(C_GB + t), None, OP.add)

            if STEPOPS < 2:
                continue
            ph1 = [ps.tile([128, 512], f32, tag=f"ph1{q}", name=f"ph1{q}") for q in range(2)]
            for q in range(2):
                for p in range(4):
                    mm(ph1[q], _b_of(p, q), wkb(K_W1, p), B32(XY[q], p),
                       True, True, p)
            if STEPOPS < 3:
                continue
            # sigma*dw into PST (diagonal tile, serialized with state MMs)
            for q in range(2):
                for p in range(4):
                    mm(PST[q], p,
                       ckb[32 * p: 32 * p + 1, K_SGR: K_SGR + 32],
                       pdwt[32 * p: 32 * p + 1, 512 * q: 512 * q + 512],
                       False, False, p)
            if STEPOPS < 4:
                continue
            # SCA = XY * dw (bf16 2x); then drift/diffusion-A into PST
            for q in range(2):
                nc.vector.tensor_tensor(out=SCA[q][:, :], in0=XY[q][:, :],
                                        in1=pdwt[:, 512 * q: 512 * q + 512],
                                        op=OP.mult)
            if STEPOPS >= 5:
                for q in range(2):
                    for p in range(4):
                        mm(PST[q], p, wkb(K_WDXY, p), B32(XY[q], p),
                           False, False, p)
                        mm(PST[q], p, wkb(K_WDSA, p), B32(SCA[q], p),
                           False, False, p)
            if STEPOPS < 6:
                continue
            filler(1)

            for q in range(2):
                nc.scalar.activation(out=H1[q][:, :], in_=ph1[q][:, :],
                                     func=AF.Tanh, bias=bias(C_B1 + t))
            ph2 = [ps.tile([128, 512], f32, tag=f"ph1{q}", name=f"ph2{q}") for q in range(2)]
            for q in range(2):
                for p in range(4):
                    b = _b_of(p, q)
                    mm(ph2[q], p, wkb(K_W2, b), B32(H1[q], b), True, True, b)
            for q in range(2):
                nc.scalar.activation(out=H2[q][:, :], in_=ph2[q][:, :],
                                     func=AF.Tanh, bias=bias(C_B2))
            if STEPOPS < 7:
                continue
            filler(1)
            pzu = [ps.tile([128, 512], f32, tag=f"pzu{q}", name=f"pzu{q}") for q in range(2)]
            for q in range(2):
                for p in range(4):
                    mm(pzu[q], p, wkb(K_W3, p), B32(H2[q], p), True, False, p)
                    mm(pzu[q], p, wkb(K_WZY, p), B32(XY[q], p),
                       False, True, p)
            if STEPOPS < 8:
                continue
            filler(1)
            # ZU = pzu + b3: stream 0 scalar, stream 1 vector
            nc.scalar.activation(out=ZU[0][:, :], in_=pzu[0][:, :],
                                 func=AF.Identity, bias=bias(C_B3))
            nc.vector.tensor_scalar(ZU[1][:, :], pzu[1][:, :],
                                    bias(C_B3), None, OP.add)
            if STEPOPS < 9:
                continue
            for q in range(2):
                nc.vector.tensor_tensor(out=SCB[q][:, :], in0=ZU[q][:, :],
                                        in1=pdwt[:, 512 * q: 512 * q + 512],
                                        op=OP.mult)
            for q in range(2):
                nc.vector.scalar_tensor_tensor(
                    out=SCR[q][:, :], in0=ZU[q][:, :], scalar=float(_ct(t)),
                    in1=ZU[q][:, :], op0=OP.mult, op1=OP.mult,
                    accum_out=lacc[:, 2 * t + q: 2 * t + q + 1])
            if STEPOPS < 10:
                continue
            for q in range(2):
                for p in range(4):
                    mm(PST[q], p, wkb(K_WDZU, p), B32(ZU[q], p),
                       False, False, p)
                    mm(PST[q], p, wkb(K_WDSB, p), B32(SCB[q], p),
                       False, last and p == 3, p)
            filler(2)

        # ---- final: loss_bsde = sum((Y - X)^2) ----
        pe = [ps.tile([128, 512], f32, tag=f"ph1{q}", name=f"ph1{q}") for q in range(2)]
        for q in range(2):
            nc.scalar.activation(out=XY[q][:, :], in_=PST[q][:, :],
                                 func=AF.Identity, bias=bias(C_GB + t_steps))
        for q in range(2):
            for p in range(4):
                mm(pe[q], p, wkb(K_WE, p), B32(XY[q], p), True, True, p)
        for q in range(2):
            nc.scalar.activation(out=SCR[q][:, :], in_=pe[q][:, :],
                                 func=AF.Square,
                                 accum_out=eacc[:, q: q + 1])
        nc.sync.dma_start(out=lacc_d[:, :], in_=lacc[:, :])
        nc.sync.dma_start(out=eacc_d[:, :], in_=eacc[:, :])

    nc.compile()
    _BUILT[t_steps] = nc
    return nc


def make_in_maps(inputs):
    ckb = pack_weights_bf16(inputs)
    ckf = pack_weights_f32(inputs)
    X0 = np.asarray(inputs["X0"], F32)
    dw = np.asarray(inputs["dw"], F32)
    in_maps = []
    for k in range(NCORES):
        in_maps.append({
            "pdw": pack_pdw(dw, k),
            "x0p": pack_x0(X0, k),
            "ckb": ckb,
            "ckf": ckf,
        })
    return in_maps


def reduce_outputs(laccs, eaccs, t_steps=T):
    dh_rows = np.zeros(128, bool)
    e_rows = np.zeros(128, bool)
    for p in range(4):
        dh_rows[32 * p + 24: 32 * p + 32] = True
        e_rows[32 * p: 32 * p + 16] = True
    lc = 0.0
    lb = 0.0
    for lacc, eacc in zip(laccs, eaccs):
        lc += float(np.sum(np.asarray(lacc, np.float64)[dh_rows,
                                                        : 2 * t_steps]))
        lb += float(np.sum(np.asarray(eacc, np.float64)[e_rows, 0:2]))
    return np.array([lb / BATCH, lc / BATCH], F32)


def kernel(**inputs):
    from concourse.bass_utils import run_bass_kernel_spmd

    nc = build(T)
    in_maps = make_in_maps(inputs)
    res = run_bass_kernel_spmd(nc, in_maps, core_ids=list(range(NCORES)))
    laccs = [r["out_lacc"] for r in res.results]
    eaccs = [r["out_eacc"] for r in res.results]
    return reduce_outputs(laccs, eaccs)


if __name__ == "__main__":
    print("module ok")


# revision 10
# speedup vs baseline: 1.0920x; 1.0920x over previous
"""Trainium2 Bass kernel for nn_DeepModel_70703751626759 (deep-BSDE forward sim).

v3: two software-pipelined sample streams (column halves) so TensorE
never idles (HAM stays warm at 2.4 GHz); dw broadcast moved off the PE
onto DMA (host-replicated rows) so SCA/SCB are bf16 2x-mode DVE ops;
rotated PE tile placement (H1 at p+-1, H2 at p+2, SGR rhs at p+2) to
spread matmuls across the 4x4 sub-array grid; elementwise work split
across ScalarE (state copy + 2 tanh) and VectorE (biased ZU copy,
SCA/SCB, loss accum).

Data-parallel over 8 NeuronCores: 32768 samples -> 4096/core -> 8 column
blocks of 512 samples; block j = p + 4q at partitions [32p, 32p+32),
free cols [512q, 512q+512). Slot content: state = [X(16); Y(16)],
ZU = [u(8); Zv(16); dH(8)]. t folded into layer-1 biases; losses
reduced on-device via accum_out.
"""

import sys
import os
import numpy as np

if "/opt/trn_rl_repo" not in sys.path:
    sys.path.insert(0, "/opt/trn_rl_repo")

N = 16
M = 8
T = 50
DT = 0.01
GAMMA = 0.1
SIGMA = 0.2
TAU = 0.5
H = 10
BATCH = 32768
NCORES = 8
CB = BATCH // NCORES
BK = 512

F32 = np.float32
try:
    import ml_dtypes
    BF16 = ml_dtypes.bfloat16
except ImportError:          # pragma: no cover
    BF16 = np.float32

# bf16 consts image (ckb) column offsets
K_W1 = 0
K_W2 = 32
K_W3 = 64
K_WZY = 96
K_WDXY = 128
K_WDZU = 160
K_WDSA = 192
K_WDSB = 224
K_WE = 256
K_WY1 = 288
K_WY2 = 320
K_WY3 = 352
K_SGR = 384
K_BY3V = 416
K_ONES512 = 448
K_COLS = 1472

# fp32 consts image (ck) column offsets
C_WYP = 0
C_B1 = 32          # 64 cols (t = 0..T-1)
C_B2 = 96
C_B3 = 97
C_BY1 = 98
C_BY2 = 99
C_GB = 100         # 51 cols: t*gamma*dt on X rows (t = 0..T)
C_COLS = 160


def _ct(t):
    w = 1.0 if (t == 0 or t == T - 1) else 2.0
    return 0.5 * DT * w * TAU * TAU


def _b_of(p, q):
    return (p + (1 if q == 0 else 3)) % 4


def _weight_blocks(inp):
    A = np.asarray(inp["A"], F32)
    Bm = np.asarray(inp["Bmat"], F32)
    C = np.asarray(inp["Cmat"], F32)
    D = np.asarray(inp["Dmat"], F32)
    ZW1 = np.asarray(inp["Z_W1"], F32)
    ZW2 = np.asarray(inp["Z_W2"], F32)
    ZW3 = np.asarray(inp["Z_W3"], F32)
    PW1 = np.asarray(inp["phi_W1"], F32)
    PW2 = np.asarray(inp["phi_W2"], F32)
    PW3 = np.asarray(inp["phi_W3"], F32)
    YW1 = np.asarray(inp["Y0_W1"], F32)
    YW2 = np.asarray(inp["Y0_W2"], F32)
    YW3 = np.asarray(inp["Y0_W3"], F32)
    I16 = np.eye(16, dtype=F32)

    def blk():
        return np.zeros((32, 32), F32)

    W1 = blk()
    W1[0:16, 0:10] = ZW1[1:, :]
    W1[0:16, 10:20] = PW1[1:, :]
    W2 = blk()
    W2[0:10, 0:10] = ZW2
    W2[10:20, 10:20] = PW2
    W3 = blk()
    W3[10:20, 0:8] = PW3
    W3[0:10, 8:24] = ZW3
    W3[0:10, 24:32] = ZW3 @ D
    W3[10:20, 24:32] = PW3
    WZY = blk()
    WZY[16:32, 24:32] = Bm
    WDXY = blk()
    WDXY[0:16, 0:16] = DT * A.T
    WDXY[0:16, 16:32] = -DT * I16
    WDXY[16:32, 16:32] = -DT * A
    WDZU = blk()
    WDZU[0:8, 0:16] = DT * Bm.T
    WDZU[8:24, 16:32] = -DT * C
    WDSA = blk()
    WDSA[0:16, 0:16] = C.T
    WDSB = blk()
    WDSB[0:8, 0:16] = D.T
    WDSB[8:24, 16:32] = I16
    WE = blk()
    WE[0:16, 0:16] = -I16
    WE[16:32, 0:16] = I16
    WY1 = blk()
    WY1[0:16, 0:10] = YW1
    WY2 = blk()
    WY2[0:10, 0:10] = YW2
    WY3 = blk()
    WY3[0:10, 16:32] = YW3
    WYP = blk()
    WYP[0:16, 0:16] = I16
    SGR = np.zeros((1, 32), F32)      # K=1 lhsT row: sigma into X rows
    SGR[0, 0:16] = SIGMA
    GDT = np.zeros((1, 32), F32)      # K=1 lhsT row: gamma*dt into X rows
    GDT[0, 0:16] = GAMMA * DT
    BY3V = np.zeros((1, 32), F32)
    BY3V[0, 16:32] = np.asarray(inp["Y0_b3"], F32)
    return dict(W1=W1, W2=W2, W3=W3, WZY=WZY, WDXY=WDXY, WDZU=WDZU,
                WDSA=WDSA, WDSB=WDSB, WE=WE, WY1=WY1, WY2=WY2, WY3=WY3,
                WYP=WYP, SGR=SGR, GDT=GDT, BY3V=BY3V)


def pack_weights_bf16(inp):
    wb = _weight_blocks(inp)
    img = np.zeros((128, K_COLS), F32)
    reps = [(K_W1, "W1"), (K_W2, "W2"), (K_W3, "W3"), (K_WZY, "WZY"),
            (K_WDXY, "WDXY"), (K_WDZU, "WDZU"), (K_WDSA, "WDSA"),
            (K_WDSB, "WDSB"), (K_WE, "WE"), (K_WY1, "WY1"),
            (K_WY2, "WY2"), (K_WY3, "WY3")]
    for p in range(4):
        r = 32 * p
        for off, name in reps:
            img[r: r + 32, off: off + 32] = wb[name]
        img[r: r + 1, K_BY3V: K_BY3V + 32] = wb["BY3V"]
        img[r: r + 1, K_SGR: K_SGR + 32] = wb["SGR"]
    img[:, K_ONES512: K_ONES512 + 1024] = 1.0
    return img.astype(BF16)


def pack_weights_f32(inp):
    wb = _weight_blocks(inp)
    Zb1 = np.asarray(inp["Z_b1"], F32)
    Zb2 = np.asarray(inp["Z_b2"], F32)
    Zb3 = np.asarray(inp["Z_b3"], F32)
    Pb1 = np.asarray(inp["phi_b1"], F32)
    Pb2 = np.asarray(inp["phi_b2"], F32)
    Pb3 = np.asarray(inp["phi_b3"], F32)
    Yb1 = np.asarray(inp["Y0_b1"], F32)
    Yb2 = np.asarray(inp["Y0_b2"], F32)
    ZW1 = np.asarray(inp["Z_W1"], F32)
    PW1 = np.asarray(inp["phi_W1"], F32)
    D = np.asarray(inp["Dmat"], F32)
    img = np.zeros((128, C_COLS), F32)
    for t in range(T):
        tv = F32(t * DT)
        b = np.concatenate([Zb1 + tv * ZW1[0, :], Pb1 + tv * PW1[0, :]])
        for p in range(4):
            img[32 * p: 32 * p + 20, C_B1 + t] = b
    b2 = np.concatenate([Zb2, Pb2])
    b3 = np.concatenate([Pb3, Zb3, Pb3 + D.T @ Zb3])
    for p in range(4):
        r = 32 * p
        img[r: r + 32, C_WYP: C_WYP + 32] = wb["WYP"]
        img[r: r + 20, C_B2] = b2
        img[r: r + 32, C_B3] = b3
        img[r: r + 10, C_BY1] = Yb1
        img[r: r + 10, C_BY2] = Yb2
        for t in range(T + 1):
            img[r: r + 16, C_GB + t] = t * GAMMA * DT
    return img


def pack_x0(X0, core):
    out = np.zeros((128, 1024), F32)
    base = core * CB
    for p in range(4):
        for q in range(2):
            j = p + 4 * q
            out[32 * p: 32 * p + 16, 512 * q: 512 * (q + 1)] = \
                X0[base + BK * j: base + BK * (j + 1), :].T
    return np.ascontiguousarray(out)


def pack_pdw(dw, core):
    """[T*128, 1024] bf16: rows [128t+32p, +32) cols [512q, +512) =
    dw of block j = p+4q at step t, replicated across the 32 rows."""
    base = core * CB
    d = np.empty((T, 4, 2, BK), F32)
    for p in range(4):
        for q in range(2):
            j = p + 4 * q
            d[:, p, q, :] = dw[:, base + BK * j: base + BK * (j + 1), 0]
    # [T, 4, 2, BK] -> [T, 4p, 32rep, 2q*BK]
    out = np.broadcast_to(d[:, :, None, :, :], (T, 4, 32, 2, BK))
    out = out.reshape(T * 128, 1024)
    return np.ascontiguousarray(out.astype(BF16))


# ---------------------------------------------------------------------------
# device program
# ---------------------------------------------------------------------------

_BUILT = {}
STEPOPS = int(os.environ.get("K_STEPOPS", "10"))


def build(t_steps=T):
    if t_steps in _BUILT:
        return _BUILT[t_steps]
    from contextlib import ExitStack
    import concourse.tile as tile
    from concourse import bacc, mybir

    f32 = mybir.dt.float32
    bf16 = mybir.dt.bfloat16
    AF = mybir.ActivationFunctionType
    OP = mybir.AluOpType

    PF = 3                     # pdw prefetch depth (steps ahead)

    nc = bacc.Bacc("TRN2", target_bir_lowering=False, debug=False)
    pdw_d = nc.dram_tensor("pdw", [T * 128, 1024], bf16,
                           kind="ExternalInput").ap()
    x0p_d = nc.dram_tensor("x0p", [128, 1024], f32, kind="ExternalInput").ap()
    ckb_d = nc.dram_tensor("ckb", [128, K_COLS], bf16,
                           kind="ExternalInput").ap()
    ckf_d = nc.dram_tensor("ckf", [128, C_COLS], f32,
                           kind="ExternalInput").ap()
    lacc_d = nc.dram_tensor("out_lacc", [128, 64], f32,
                            kind="ExternalOutput").ap()
    eacc_d = nc.dram_tensor("out_eacc", [128, 8], f32,
                            kind="ExternalOutput").ap()

    def B32(tens, g):
        return tens[32 * g: 32 * g + 32, :]

    with tile.TileContext(nc) as tc, ExitStack() as ctx:
        sb = ctx.enter_context(tc.tile_pool(name="sb", bufs=1))
        dwpool = ctx.enter_context(tc.tile_pool(name="dwp", bufs=PF + 1))
        ps = ctx.enter_context(tc.tile_pool(name="ps", bufs=1, space="PSUM"))

        ckb = sb.tile([128, K_COLS], bf16, tag="ckb")
        ckf = sb.tile([128, C_COLS], f32, tag="ckf")
        nc.sync.dma_start(out=ckb[:, :], in_=ckb_d[:, :])
        nc.sync.dma_start(out=ckf[:, :], in_=ckf_d[:, :])
        X0SB = sb.tile([128, 1024], f32, tag="X0SB")
        nc.sync.dma_start(out=X0SB[:, :], in_=x0p_d[:, :])

        XY = [sb.tile([128, 512], bf16, tag=f"XY{q}", name=f"XY{q}") for q in range(2)]
        H1 = [sb.tile([128, 512], bf16, tag=f"H1{q}", name=f"H1{q}") for q in range(2)]
        H2 = [sb.tile([128, 512], bf16, tag=f"H2{q}", name=f"H2{q}") for q in range(2)]
        ZUm = sb.tile([128, 1024], bf16, tag="ZUm", name="ZUm")
        SCA = [sb.tile([128, 512], bf16, tag=f"SCA{q}", name=f"SCA{q}") for q in range(2)]
        SCB = [sb.tile([128, 512], bf16, tag=f"SCB{q}", name=f"SCB{q}") for q in range(2)]
        SCRm = sb.tile([128, 1024], bf16, tag="SCRm", name="SCRm")
        lacc = sb.tile([128, 64], f32, tag="lacc")
        eacc = sb.tile([128, 8], f32, tag="eacc")
        nc.vector.memset(lacc[:, :], 0.0)
        nc.vector.memset(eacc[:, :], 0.0)

        PST = [ps.tile([128, 512], f32, tag=f"pst{q}", name=f"pst{q}") for q in range(2)]
        DUM = ps.tile([128, 512], f32, tag="dum", name="DUM")
        _fill_g = [0]
        NDUM = int(os.environ.get("K_DUMMY", "1"))

        def filler(n=1):
            # Keep the PE array streaming through dependency stalls so the
            # HAM clock gate stays at 8/8. Runs on the otherwise-unused
            # (g, g+2) tile set into a scratch PSUM bank.
            for _ in range(n * NDUM):
                g = _fill_g[0] = (_fill_g[0] + 1) % 4
                d = (g + 2) % 4
                nc.tensor.matmul(
                    out=DUM[32 * d: 32 * d + 32, :],
                    lhsT=wkb(K_W1, g),
                    rhs=ckb[32 * g: 32 * g + 32, K_ONES512: K_ONES512 + 512],
                    start=True, stop=True,
                    tile_position=(32 * g, 32 * d),
                    skip_group_check=True)

        def wkb(off, g):
            return ckb[32 * g: 32 * g + 32, off: off + 32]

        def wkf(off, g):
            return ckf[32 * g: 32 * g + 32, off: off + 32]

        def bias(col):
            return ckf[:, col: col + 1]

        def mm(out_t, og, lhsT, rhs, start, stop, rg):
            nc.tensor.matmul(out=B32(out_t, og), lhsT=lhsT, rhs=rhs,
                             start=start, stop=stop,
                             tile_position=(32 * rg, 32 * og),
                             skip_group_check=True)

        # ---- warm-up burst: ~8.5us of back-to-back PE work ----
        filler(10)

        # ---- init: PST = WYP @ X0; PST[Y rows] += Y0-MLP(X0) ----
        for q in range(2):
            for p in range(4):
                mm(PST[q], p, wkf(C_WYP, p),
                   X0SB[32 * p: 32 * p + 32, 512 * q: 512 * q + 512],
                   True, False, p)
        ph_i = [ps.tile([128, 512], f32, tag=f"ph1{q}", name=f"ph1{q}") for q in range(2)]
        ph2_i = [ps.tile([128, 512], f32, tag=f"ph1{q}", name=f"ph2i{q}") for q in range(2)]
        for q in range(2):
            nc.scalar.activation(out=XY[q][:, :], in_=PST[q][:, :],
                                 func=AF.Copy)
        for q in range(2):
            for p in range(4):
                mm(ph_i[q], _b_of(p, q), wkb(K_WY1, p), B32(XY[q], p),
                   True, True, p)
        for q in range(2):
            nc.scalar.activation(out=H1[q][:, :], in_=ph_i[q][:, :],
                                 func=AF.Tanh, bias=bias(C_BY1))
        for q in range(2):
            for p in range(4):
                b = _b_of(p, q)
                mm(ph2_i[q], p, wkb(K_WY2, b), B32(H1[q], b), True, True, b)
        for q in range(2):
            nc.scalar.activation(out=H2[q][:, :], in_=ph2_i[q][:, :],
                                 func=AF.Tanh, bias=bias(C_BY2))
        for q in range(2):
            for p in range(4):
                mm(PST[q], p, wkb(K_WY3, p), B32(H2[q], p), False, False, p)
                mm(PST[q], p, ckb[32 * p: 32 * p + 1, K_BY3V: K_BY3V + 32],
                   ckb[32 * p: 32 * p + 1, K_ONES512: K_ONES512 + 512],
                   False, False, p)

        # ---- pdw prefetch for first PF steps ----
        pdw_tiles = {}

        def fetch_pdw(t):
            pt = dwpool.tile([128, 1024], bf16, tag="pdw", name="pdwt")
            nc.sync.dma_start(out=pt[:, :],
                              in_=pdw_d[128 * t: 128 * (t + 1), :])
            pdw_tiles[t] = pt

        for t in range(min(PF, t_steps)):
            fetch_pdw(t)

        # ---- steps ----
        for t in range(t_steps):
            if t + PF < t_steps:
                fetch_pdw(t + PF)
            pdwt = pdw_tiles.pop(t)
            last = (t == t_steps - 1)

            # state copy (+ t*gamma*dt on X rows): scalar / vector
            nc.scalar.activation(out=XY[0][:, :], in_=PST[0][:, :],
                                 func=AF.Identity, bias=bias(C_GB + t))
            nc.vector.tensor_scalar(XY[1][:, :], PST[1][:, :],
                                    bias(C_GB + t), None, OP.add)
            filler(2)

            ph1 = [ps.tile([128, 512], f32, tag=f"ph1{q}", name=f"ph1{q}")
                   for q in range(2)]
            pzu = [ps.tile([128, 512], f32, tag=f"pzu{q}", name=f"pzu{q}")
                   for q in range(2)]
            for q in range(2):
                for p in range(4):
                    mm(ph1[q], _b_of(p, q), wkb(K_W1, p), B32(XY[q], p),
                       True, True, p)
            # sigma*dw into PST; WZY opens the pzu accumulation early
            for q in range(2):
                for p in range(4):
                    mm(PST[q], p,
                       ckb[32 * p: 32 * p + 1, K_SGR: K_SGR + 32],
                       pdwt[32 * p: 32 * p + 1, 512 * q: 512 * q + 512],
                       False, False, p)
                    mm(pzu[q], p, wkb(K_WZY, p), B32(XY[q], p),
                       True, False, p)
            # SCA = XY * dw (vector, off critical path)
            for q in range(2):
                nc.vector.tensor_tensor(out=SCA[q][:, :], in0=XY[q][:, :],
                                        in1=pdwt[:, 512 * q: 512 * q + 512],
                                        op=OP.mult)
            filler(1)

            for q in range(2):
                nc.scalar.activation(out=H1[q][:, :], in_=ph1[q][:, :],
                                     func=AF.Tanh, bias=bias(C_B1 + t))
            ph2 = [ps.tile([128, 512], f32, tag=f"ph1{q}", name=f"ph2{q}")
                   for q in range(2)]
            for q in range(2):
                for p in range(4):
                    b = _b_of(p, q)
                    mm(ph2[q], p, wkb(K_W2, b), B32(H1[q], b), True, True, b)
            for q in range(2):
                for p in range(4):
                    mm(PST[q], p, wkb(K_WDXY, p), B32(XY[q], p),
                       False, False, p)
                    mm(PST[q], p, wkb(K_WDSA, p), B32(SCA[q], p),
                       False, False, p)
            filler(2)

            for q in range(2):
                nc.scalar.activation(out=H2[q][:, :], in_=ph2[q][:, :],
                                     func=AF.Tanh, bias=bias(C_B2))
            for q in range(2):
                for p in range(4):
                    mm(pzu[q], p, wkb(K_W3, p), B32(H2[q], p), False, True, p)
            filler(2)

            # SCB = (pzu + b3) * dw directly (fused); ZU copies in parallel
            nc.scalar.activation(out=ZUm[:, 0:512], in_=pzu[0][:, :],
                                 func=AF.Identity, bias=bias(C_B3))
            nc.vector.scalar_tensor_tensor(
                out=SCB[0][:, :], in0=pzu[0][:, :], scalar=bias(C_B3),
                in1=pdwt[:, 0:512], op0=OP.add, op1=OP.mult)
            nc.vector.tensor_scalar(ZUm[:, 512:1024], pzu[1][:, :],
                                    bias(C_B3), None, OP.add)
            nc.vector.scalar_tensor_tensor(
                out=SCB[1][:, :], in0=pzu[1][:, :], scalar=bias(C_B3),
                in1=pdwt[:, 512:1024], op0=OP.add, op1=OP.mult)
            for q in range(2):
                for p in range(4):
                    mm(PST[q], p, wkb(K_WDSB, p), B32(SCB[q], p),
                       False, False, p)
                    mm(PST[q], p, wkb(K_WDZU, p),
                       ZUm[32 * p: 32 * p + 32, 512 * q: 512 * q + 512],
                       False, last and q == 1 and p == 3, p)
            # loss accum over both halves at once
            nc.vector.scalar_tensor_tensor(
                out=SCRm[:, :], in0=ZUm[:, :], scalar=float(_ct(t)),
                in1=ZUm[:, :], op0=OP.mult, op1=OP.mult,
                accum_out=lacc[:, t: t + 1])
            filler(2)

        # ---- final: loss_bsde = sum((Y - X)^2) ----
        pe = [ps.tile([128, 512], f32, tag=f"ph1{q}", name=f"ph1{q}") for q in range(2)]
        for q in range(2):
            nc.scalar.activation(out=XY[q][:, :], in_=PST[q][:, :],
                                 func=AF.Identity, bias=bias(C_GB + t_steps))
        for q in range(2):
            for p in range(4):
                mm(pe[q], p, wkb(K_WE, p), B32(XY[q], p), True, True, p)
        for q in range(2):
            nc.scalar.activation(out=SCRm[:, 512 * q: 512 * q + 512],
                                 in_=pe[q][:, :], func=AF.Square,
                                 accum_out=eacc[:, q: q + 1])
        nc.sync.dma_start(out=lacc_d[:, :], in_=lacc[:, :])
        nc.sync.dma_start(out=eacc_d[:, :], in_=eacc[:, :])

    nc.compile()
    _BUILT[t_steps] = nc
    return nc


def make_in_maps(inputs):
    ckb = pack_weights_bf16(inputs)
    ckf = pack_weights_f32(inputs)
    X0 = np.asarray(inputs["X0"], F32)
    dw = np.asarray(inputs["dw"], F32)
    in_maps = []
    for k in range(NCORES):
        in_maps.append({
            "pdw": pack_pdw(dw, k),
            "x0p": pack_x0(X0, k),
            "ckb": ckb,
            "ckf": ckf,
        })
    return in_maps


def reduce_outputs(laccs, eaccs, t_steps=T):
    dh_rows = np.zeros(128, bool)
    e_rows = np.zeros(128, bool)
    for p in range(4):
        dh_rows[32 * p + 24: 32 * p + 32] = True
        e_rows[32 * p: 32 * p + 16] = True
    lc = 0.0
    lb = 0.0
    for lacc, eacc in zip(laccs, eaccs):
        lc += float(np.sum(np.asarray(lacc, np.float64)[dh_rows, :t_steps]))
        lb += float(np.sum(np.asarray(eacc, np.float64)[e_rows, 0:2]))
    return np.array([lb / BATCH, lc / BATCH], F32)


def kernel(**inputs):
    from concourse.bass_utils import run_bass_kernel_spmd

    nc = build(T)
    in_maps = make_in_maps(inputs)
    res = run_bass_kernel_spmd(nc, in_maps, core_ids=list(range(NCORES)))
    laccs = [r["out_lacc"] for r in res.results]
    eaccs = [r["out_eacc"] for r in res.results]
    return reduce_outputs(laccs, eaccs)


if __name__ == "__main__":
    print("module ok")


# revision 12
# speedup vs baseline: 1.3831x; 1.2666x over previous
"""Trainium2 Bass kernel for nn_DeepModel_70703751626759 (deep-BSDE forward sim).

v3: two software-pipelined sample streams (column halves) so TensorE
never idles (HAM stays warm at 2.4 GHz); dw broadcast moved off the PE
onto DMA (host-replicated rows) so SCA/SCB are bf16 2x-mode DVE ops;
rotated PE tile placement (H1 at p+-1, H2 at p+2, SGR rhs at p+2) to
spread matmuls across the 4x4 sub-array grid; elementwise work split
across ScalarE (state copy + 2 tanh) and VectorE (biased ZU copy,
SCA/SCB, loss accum).

Data-parallel over 8 NeuronCores: 32768 samples -> 4096/core -> 8 column
blocks of 512 samples; block j = p + 4q at partitions [32p, 32p+32),
free cols [512q, 512q+512). Slot content: state = [X(16); Y(16)],
ZU = [u(8); Zv(16); dH(8)]. t folded into layer-1 biases; losses
reduced on-device via accum_out.
"""

import sys
import os
import numpy as np

if "/opt/trn_rl_repo" not in sys.path:
    sys.path.insert(0, "/opt/trn_rl_repo")

N = 16
M = 8
T = 50
DT = 0.01
GAMMA = 0.1
SIGMA = 0.2
TAU = 0.5
H = 10
BATCH = 32768
NCORES = 8
CB = BATCH // NCORES
BK = 512

F32 = np.float32
try:
    import ml_dtypes
    BF16 = ml_dtypes.bfloat16
except ImportError:          # pragma: no cover
    BF16 = np.float32

# bf16 consts image (ckb) column offsets
K_W1 = 0
K_W2 = 32
K_W3 = 64
K_WZY = 96
K_WDXY = 128
K_WDZU = 160
K_WDSA = 192
K_WDSB = 224
K_WE = 256
K_WY1 = 288
K_WY2 = 320
K_WY3 = 352
K_SGR = 384
K_BY3V = 416
K_ONES512 = 448
K_W1X = 1472
K_W1SA = 1504
K_W1ZU = 1536
K_W1SB = 1568
K_SGW = 1600
K_COLS = 1632

# fp32 consts image (ck) column offsets
C_WYP = 0
C_B1 = 32          # 64 cols (t = 0..T-1)
C_B2 = 96
C_B3 = 97
C_BY1 = 98
C_BY2 = 99
C_GB = 100         # 51 cols: t*gamma*dt on X rows (t = 0..T)
C_COLS = 160


def _ct(t):
    w = 1.0 if (t == 0 or t == T - 1) else 2.0
    return 0.5 * DT * w * TAU * TAU


def _b_of(p, q):
    return (p + (1 if q == 0 else 3)) % 4


def _weight_blocks(inp):
    A = np.asarray(inp["A"], F32)
    Bm = np.asarray(inp["Bmat"], F32)
    C = np.asarray(inp["Cmat"], F32)
    D = np.asarray(inp["Dmat"], F32)
    ZW1 = np.asarray(inp["Z_W1"], F32)
    ZW2 = np.asarray(inp["Z_W2"], F32)
    ZW3 = np.asarray(inp["Z_W3"], F32)
    PW1 = np.asarray(inp["phi_W1"], F32)
    PW2 = np.asarray(inp["phi_W2"], F32)
    PW3 = np.asarray(inp["phi_W3"], F32)
    YW1 = np.asarray(inp["Y0_W1"], F32)
    YW2 = np.asarray(inp["Y0_W2"], F32)
    YW3 = np.asarray(inp["Y0_W3"], F32)
    I16 = np.eye(16, dtype=F32)

    def blk():
        return np.zeros((32, 32), F32)

    W1 = blk()
    W1[0:16, 0:10] = ZW1[1:, :]
    W1[0:16, 10:20] = PW1[1:, :]
    W2 = blk()
    W2[0:10, 0:10] = ZW2
    W2[10:20, 10:20] = PW2
    W3 = blk()
    W3[10:20, 0:8] = PW3
    W3[0:10, 8:24] = ZW3
    W3[0:10, 24:32] = ZW3 @ D
    W3[10:20, 24:32] = PW3
    WZY = blk()
    WZY[16:32, 24:32] = Bm
    WDXY = blk()
    WDXY[0:16, 0:16] = DT * A.T
    WDXY[0:16, 16:32] = -DT * I16
    WDXY[16:32, 16:32] = -DT * A
    WDZU = blk()
    WDZU[0:8, 0:16] = DT * Bm.T
    WDZU[8:24, 16:32] = -DT * C
    WDSA = blk()
    WDSA[0:16, 0:16] = C.T
    WDSB = blk()
    WDSB[0:8, 0:16] = D.T
    WDSB[8:24, 16:32] = I16
    WE = blk()
    WE[0:16, 0:16] = -I16
    WE[16:32, 0:16] = I16
    WY1 = blk()
    WY1[0:16, 0:10] = YW1
    WY2 = blk()
    WY2[0:10, 0:10] = YW2
    WY3 = blk()
    WY3[0:10, 16:32] = YW3
    WYP = blk()
    WYP[0:16, 0:16] = I16
    SGR = np.zeros((1, 32), F32)      # K=1 lhsT row: sigma into X rows
    SGR[0, 0:16] = SIGMA
    GDT = np.zeros((1, 32), F32)      # K=1 lhsT row: gamma*dt into X rows
    GDT[0, 0:16] = GAMMA * DT
    BY3V = np.zeros((1, 32), F32)
    BY3V[0, 16:32] = np.asarray(inp["Y0_b3"], F32)
    W1X = W1 + WDXY @ W1
    W1SA = WDSA @ W1
    W1ZU = WDZU @ W1
    W1SB = WDSB @ W1
    SGW = np.zeros((1, 32), F32)
    SGW[0, :] = SIGMA * W1[0:16, :].sum(axis=0)
    return dict(W1=W1, W2=W2, W3=W3, WZY=WZY, WDXY=WDXY, WDZU=WDZU,
                WDSA=WDSA, WDSB=WDSB, WE=WE, WY1=WY1, WY2=WY2, WY3=WY3,
                WYP=WYP, SGR=SGR, GDT=GDT, BY3V=BY3V, W1X=W1X, W1SA=W1SA,
                W1ZU=W1ZU, W1SB=W1SB, SGW=SGW)


def pack_weights_bf16(inp):
    wb = _weight_blocks(inp)
    img = np.zeros((128, K_COLS), F32)
    reps = [(K_W1, "W1"), (K_W2, "W2"), (K_W3, "W3"), (K_WZY, "WZY"),
            (K_WDXY, "WDXY"), (K_WDZU, "WDZU"), (K_WDSA, "WDSA"),
            (K_WDSB, "WDSB"), (K_WE, "WE"), (K_WY1, "WY1"),
            (K_WY2, "WY2"), (K_WY3, "WY3"), (K_W1X, "W1X"),
            (K_W1SA, "W1SA"), (K_W1ZU, "W1ZU"), (K_W1SB, "W1SB")]
    for p in range(4):
        r = 32 * p
        for off, name in reps:
            img[r: r + 32, off: off + 32] = wb[name]
        img[r: r + 1, K_BY3V: K_BY3V + 32] = wb["BY3V"]
        img[r: r + 1, K_SGR: K_SGR + 32] = wb["SGR"]
        img[r: r + 1, K_SGW: K_SGW + 32] = wb["SGW"]
    img[:, K_ONES512: K_ONES512 + 1024] = 1.0
    return img.astype(BF16)


def pack_weights_f32(inp):
    wb = _weight_blocks(inp)
    Zb1 = np.asarray(inp["Z_b1"], F32)
    Zb2 = np.asarray(inp["Z_b2"], F32)
    Zb3 = np.asarray(inp["Z_b3"], F32)
    Pb1 = np.asarray(inp["phi_b1"], F32)
    Pb2 = np.asarray(inp["phi_b2"], F32)
    Pb3 = np.asarray(inp["phi_b3"], F32)
    Yb1 = np.asarray(inp["Y0_b1"], F32)
    Yb2 = np.asarray(inp["Y0_b2"], F32)
    ZW1 = np.asarray(inp["Z_W1"], F32)
    PW1 = np.asarray(inp["phi_W1"], F32)
    D = np.asarray(inp["Dmat"], F32)
    img = np.zeros((128, C_COLS), F32)
    w1sum20 = wb["W1"][0:16, 0:20].sum(axis=0)
    for t in range(T):
        tv = F32(t * DT)
        b = np.concatenate([Zb1 + tv * ZW1[0, :], Pb1 + tv * PW1[0, :]])
        if t >= 1:
            b = b + GAMMA * DT * w1sum20
        for p in range(4):
            img[32 * p: 32 * p + 20, C_B1 + t] = b
    b2 = np.concatenate([Zb2, Pb2])
    b3 = np.concatenate([Pb3, Zb3, Pb3 + D.T @ Zb3])
    for p in range(4):
        r = 32 * p
        img[r: r + 32, C_WYP: C_WYP + 32] = wb["WYP"]
        img[r: r + 20, C_B2] = b2
        img[r: r + 32, C_B3] = b3
        img[r: r + 10, C_BY1] = Yb1
        img[r: r + 10, C_BY2] = Yb2
        for t in range(T + 1):
            img[r: r + 16, C_GB + t] = t * GAMMA * DT
    return img


def pack_x0(X0, core):
    out = np.zeros((128, 1024), F32)
    base = core * CB
    for p in range(4):
        for q in range(2):
            j = p + 4 * q
            out[32 * p: 32 * p + 16, 512 * q: 512 * (q + 1)] = \
                X0[base + BK * j: base + BK * (j + 1), :].T
    return np.ascontiguousarray(out)


def pack_pdw(dw, core):
    """[T*128, 1024] bf16: rows [128t+32p, +32) cols [512q, +512) =
    dw of block j = p+4q at step t, replicated across the 32 rows."""
    base = core * CB
    d = np.empty((T, 4, 2, BK), F32)
    for p in range(4):
        for q in range(2):
            j = p + 4 * q
            d[:, p, q, :] = dw[:, base + BK * j: base + BK * (j + 1), 0]
    # [T, 4, 2, BK] -> [T, 4p, 32rep, 2q*BK]
    out = np.broadcast_to(d[:, :, None, :, :], (T, 4, 32, 2, BK))
    out = out.reshape(T * 128, 1024)
    return np.ascontiguousarray(out.astype(BF16))


# ---------------------------------------------------------------------------
# device program
# ---------------------------------------------------------------------------

_BUILT = {}
STEPOPS = int(os.environ.get("K_STEPOPS", "10"))


def build(t_steps=T):
    if t_steps in _BUILT:
        return _BUILT[t_steps]
    from contextlib import ExitStack
    import concourse.tile as tile
    from concourse import bacc, mybir

    f32 = mybir.dt.float32
    bf16 = mybir.dt.bfloat16
    AF = mybir.ActivationFunctionType
    OP = mybir.AluOpType

    PF = 3                     # pdw prefetch depth (steps ahead)

    nc = bacc.Bacc("TRN2", target_bir_lowering=False, debug=False)
    pdw_d = nc.dram_tensor("pdw", [T * 128, 1024], bf16,
                           kind="ExternalInput").ap()
    x0p_d = nc.dram_tensor("x0p", [128, 1024], f32, kind="ExternalInput").ap()
    ckb_d = nc.dram_tensor("ckb", [128, K_COLS], bf16,
                           kind="ExternalInput").ap()
    ckf_d = nc.dram_tensor("ckf", [128, C_COLS], f32,
                           kind="ExternalInput").ap()
    lacc_d = nc.dram_tensor("out_lacc", [128, 64], f32,
                            kind="ExternalOutput").ap()
    eacc_d = nc.dram_tensor("out_eacc", [128, 8], f32,
                            kind="ExternalOutput").ap()

    def B32(tens, g):
        return tens[32 * g: 32 * g + 32, :]

    with tile.TileContext(nc) as tc, ExitStack() as ctx:
        sb = ctx.enter_context(tc.tile_pool(name="sb", bufs=1))
        dwpool = ctx.enter_context(tc.tile_pool(name="dwp", bufs=PF + 1))
        ps = ctx.enter_context(tc.tile_pool(name="ps", bufs=1, space="PSUM"))

        ckb = sb.tile([128, K_COLS], bf16, tag="ckb")
        ckf = sb.tile([128, C_COLS], f32, tag="ckf")
        nc.sync.dma_start(out=ckb[:, :], in_=ckb_d[:, :])
        nc.sync.dma_start(out=ckf[:, :], in_=ckf_d[:, :])
        X0SB = sb.tile([128, 1024], f32, tag="X0SB")
        nc.sync.dma_start(out=X0SB[:, :], in_=x0p_d[:, :])

        XY = [sb.tile([128, 512], bf16, tag=f"XY{q}", name=f"XY{q}") for q in range(2)]
        H1 = [sb.tile([128, 512], bf16, tag=f"H1{q}", name=f"H1{q}") for q in range(2)]
        H2 = [sb.tile([128, 512], bf16, tag=f"H2{q}", name=f"H2{q}") for q in range(2)]
        ZUm = sb.tile([128, 1024], bf16, tag="ZUm", name="ZUm")
        SCA = [sb.tile([128, 512], bf16, tag=f"SCA{q}", name=f"SCA{q}") for q in range(2)]
        SCB = [sb.tile([128, 512], bf16, tag=f"SCB{q}", name=f"SCB{q}") for q in range(2)]
        SCRm = sb.tile([128, 1024], bf16, tag="SCRm", name="SCRm")
        lacc = sb.tile([128, 64], f32, tag="lacc")
        eacc = sb.tile([128, 8], f32, tag="eacc")
        nc.vector.memset(lacc[:, :], 0.0)
        nc.vector.memset(eacc[:, :], 0.0)

        PST = [ps.tile([128, 512], f32, tag=f"pst{q}", name=f"pst{q}") for q in range(2)]

        def wkb(off, g):
            return ckb[32 * g: 32 * g + 32, off: off + 32]

        def wkf(off, g):
            return ckf[32 * g: 32 * g + 32, off: off + 32]

        def bias(col):
            return ckf[:, col: col + 1]

        def mm(out_t, og, lhsT, rhs, start, stop, rg):
            nc.tensor.matmul(out=B32(out_t, og), lhsT=lhsT, rhs=rhs,
                             start=start, stop=stop,
                             tile_position=(32 * rg, 32 * og),
                             skip_group_check=True)

        # ---- init: PST = WYP @ X0; PST[Y rows] += Y0-MLP(X0) ----
        for q in range(2):
            for p in range(4):
                mm(PST[q], p, wkf(C_WYP, p),
                   X0SB[32 * p: 32 * p + 32, 512 * q: 512 * q + 512],
                   True, False, p)
        ph_i = [ps.tile([128, 512], f32, tag=f"ph1{q}", name=f"ph1{q}") for q in range(2)]
        ph2_i = [ps.tile([128, 512], f32, tag=f"ph2{q}", name=f"ph2i{q}") for q in range(2)]
        for q in range(2):
            nc.scalar.activation(out=XY[q][:, :], in_=PST[q][:, :],
                                 func=AF.Copy)
        for q in range(2):
            for p in range(4):
                mm(ph_i[q], _b_of(p, q), wkb(K_WY1, p), B32(XY[q], p),
                   True, True, p)
        for q in range(2):
            nc.scalar.activation(out=H1[q][:, :], in_=ph_i[q][:, :],
                                 func=AF.Tanh, bias=bias(C_BY1))
        for q in range(2):
            for p in range(4):
                b = _b_of(p, q)
                mm(ph2_i[q], p, wkb(K_WY2, b), B32(H1[q], b), True, True, b)
        for q in range(2):
            nc.scalar.activation(out=H2[q][:, :], in_=ph2_i[q][:, :],
                                 func=AF.Tanh, bias=bias(C_BY2))
        for q in range(2):
            for p in range(4):
                mm(PST[q], p, wkb(K_WY3, p), B32(H2[q], p), False, False, p)
                mm(PST[q], p, ckb[32 * p: 32 * p + 1, K_BY3V: K_BY3V + 32],
                   ckb[32 * p: 32 * p + 1, K_ONES512: K_ONES512 + 512],
                   False, False, p)

        # ---- pdw prefetch for first PF steps ----
        pdw_tiles = {}

        def fetch_pdw(t):
            pt = dwpool.tile([128, 1024], bf16, tag="pdw", name="pdwt")
            nc.sync.dma_start(out=pt[:, :],
                              in_=pdw_d[128 * t: 128 * (t + 1), :])
            pdw_tiles[t] = pt

        for t in range(min(PF, t_steps)):
            fetch_pdw(t)

        # ---- steps ----
        # ph1(t) for t>=1 is accumulated during step t-1 from premultiplied
        # weights: ph1(t) = W1Xt.XY + W1SA.SCA + W1ZU.ZU + W1SB.SCB + SGW.dw
        # (gamma*dt absorbed into the tanh1 bias), so tanh1 never waits on
        # the state copy or a separate W1 matmul.
        ph1 = None
        for t in range(t_steps):
            if t + PF < t_steps:
                fetch_pdw(t + PF)
            pdwt = pdw_tiles.pop(t)
            last = (t == t_steps - 1)

            # state copy (+ t*gamma*dt on X rows) -- off critical path
            nc.scalar.activation(out=XY[0][:, :], in_=PST[0][:, :],
                                 func=AF.Identity, bias=bias(C_GB + t))
            nc.vector.tensor_scalar(XY[1][:, :], PST[1][:, :],
                                    bias(C_GB + t), None, OP.add)
            if t == 0:
                ph1 = [ps.tile([128, 512], f32, tag=f"ph1{q}",
                               name=f"ph1{q}") for q in range(2)]
                for q in range(2):
                    for p in range(4):
                        mm(ph1[q], _b_of(p, q), wkb(K_W1, p), B32(XY[q], p),
                           True, True, p)
            for q in range(2):
                nc.vector.tensor_tensor(out=SCA[q][:, :], in0=XY[q][:, :],
                                        in1=pdwt[:, 512 * q: 512 * q + 512],
                                        op=OP.mult)

            # chain head: tanh1 on the pre-accumulated ph1(t)
            for q in range(2):
                nc.scalar.activation(out=H1[q][:, :], in_=ph1[q][:, :],
                                     func=AF.Tanh, bias=bias(C_B1 + t))
            ph2 = [ps.tile([128, 512], f32, tag=f"ph2{q}", name=f"ph2{q}")
                   for q in range(2)]
            for q in range(2):
                for p in range(4):
                    b = _b_of(p, q)
                    mm(ph2[q], p, wkb(K_W2, b), B32(H1[q], b), True, True, b)
            # off-chain diag work (runs while tanh2 pends)
            pzu = [ps.tile([128, 512], f32, tag=f"pzu{q}", name=f"pzu{q}")
                   for q in range(2)]
            for q in range(2):
                for p in range(4):
                    mm(PST[q], p,
                       ckb[32 * p: 32 * p + 1, K_SGR: K_SGR + 32],
                       pdwt[32 * p: 32 * p + 1, 512 * q: 512 * q + 512],
                       False, False, p)
                    mm(pzu[q], p, wkb(K_WZY, p), B32(XY[q], p),
                       True, False, p)
                    mm(PST[q], p, wkb(K_WDXY, p), B32(XY[q], p),
                       False, False, p)
            for q in range(2):
                for p in range(4):
                    mm(PST[q], p, wkb(K_WDSA, p), B32(SCA[q], p),
                       False, False, p)

            for q in range(2):
                nc.scalar.activation(out=H2[q][:, :], in_=ph2[q][:, :],
                                     func=AF.Tanh, bias=bias(C_B2))
            for q in range(2):
                for p in range(4):
                    mm(pzu[q], p, wkb(K_W3, p), B32(H2[q], p), False, True, p)

            # ZU and SCB (fused bias+dw) -- SCB' A first: it is on the chain
            nc.vector.scalar_tensor_tensor(
                out=SCB[0][:, :], in0=pzu[0][:, :], scalar=bias(C_B3),
                in1=pdwt[:, 0:512], op0=OP.add, op1=OP.mult)
            nc.scalar.activation(out=ZUm[:, 0:512], in_=pzu[0][:, :],
                                 func=AF.Identity, bias=bias(C_B3))
            nc.vector.scalar_tensor_tensor(
                out=SCB[1][:, :], in0=pzu[1][:, :], scalar=bias(C_B3),
                in1=pdwt[:, 512:1024], op0=OP.add, op1=OP.mult)
            nc.vector.tensor_scalar(ZUm[:, 512:1024], pzu[1][:, :],
                                    bias(C_B3), None, OP.add)

            # accumulate ph1(t+1) (chain: W1SB after SCB'); then PST updates
            if not last:
                ph1 = [ps.tile([128, 512], f32, tag=f"ph1{q}",
                               name=f"ph1n{q}") for q in range(2)]
                for q in range(2):
                    for p in range(4):
                        b = _b_of(p, q)
                        mm(ph1[q], b, wkb(K_W1X, p), B32(XY[q], p),
                           True, False, p)
                        mm(ph1[q], b,
                           ckb[32 * p: 32 * p + 1, K_SGW: K_SGW + 32],
                           pdwt[32 * p: 32 * p + 1, 512 * q: 512 * q + 512],
                           False, False, p)
                        mm(ph1[q], b, wkb(K_W1SA, p), B32(SCA[q], p),
                           False, False, p)
                for q in range(2):
                    for p in range(4):
                        b = _b_of(p, q)
                        mm(ph1[q], b, wkb(K_W1SB, p), B32(SCB[q], p),
                           False, False, p)
                        mm(ph1[q], b, wkb(K_W1ZU, p),
                           ZUm[32 * p: 32 * p + 32, 512 * q: 512 * q + 512],
                           False, True, p)
            for q in range(2):
                for p in range(4):
                    mm(PST[q], p, wkb(K_WDSB, p), B32(SCB[q], p),
                       False, False, p)
                    mm(PST[q], p, wkb(K_WDZU, p),
                       ZUm[32 * p: 32 * p + 32, 512 * q: 512 * q + 512],
                       False, last and q == 1 and p == 3, p)
            nc.vector.scalar_tensor_tensor(
                out=SCRm[:, :], in0=ZUm[:, :], scalar=float(_ct(t)),
                in1=ZUm[:, :], op0=OP.mult, op1=OP.mult,
                accum_out=lacc[:, t: t + 1])

        # ---- final: loss_bsde = sum((Y - X)^2) ----
        pe = [ps.tile([128, 512], f32, tag=f"ph1{q}", name=f"ph1{q}") for q in range(2)]
        for q in range(2):
            nc.scalar.activation(out=XY[q][:, :], in_=PST[q][:, :],
                                 func=AF.Identity, bias=bias(C_GB + t_steps))
        for q in range(2):
            for p in range(4):
                mm(pe[q], p, wkb(K_WE, p), B32(XY[q], p), True, True, p)
        for q in range(2):
            nc.scalar.activation(out=SCRm[:, 512 * q: 512 * q + 512],
                                 in_=pe[q][:, :], func=AF.Square,
                                 accum_out=eacc[:, q: q + 1])
        nc.sync.dma_start(out=lacc_d[:, :], in_=lacc[:, :])
        nc.sync.dma_start(out=eacc_d[:, :], in_=eacc[:, :])

    nc.compile()
    _BUILT[t_steps] = nc
    return nc


def make_in_maps(inputs):
    ckb = pack_weights_bf16(inputs)
    ckf = pack_weights_f32(inputs)
    X0 = np.asarray(inputs["X0"], F32)
    dw = np.asarray(inputs["dw"], F32)
    in_maps = []
    for k in range(NCORES):
        in_maps.append({
            "pdw": pack_pdw(dw, k),
            "x0p": pack_x0(X0, k),
            "ckb": ckb,
            "ckf": ckf,
        })
    return in_maps


def reduce_outputs(laccs, eaccs, t_steps=T):
    dh_rows = np.zeros(128, bool)
    e_rows = np.zeros(128, bool)
    for p in range(4):
        dh_rows[32 * p + 24: 32 * p + 32] = True
        e_rows[32 * p: 32 * p + 16] = True
    lc = 0.0
    lb = 0.0
    for lacc, eacc in zip(laccs, eaccs):
        lc += float(np.sum(np.asarray(lacc, np.float64)[dh_rows, :t_steps]))
        lb += float(np.sum(np.asarray(eacc, np.float64)[e_rows, 0:2]))
    return np.array([lb / BATCH, lc / BATCH], F32)


def kernel(**inputs):
    from concourse.bass_utils import run_bass_kernel_spmd

    nc = build(T)
    in_maps = make_in_maps(inputs)
    res = run_bass_kernel_spmd(nc, in_maps, core_ids=list(range(NCORES)))
    laccs = [r["out_lacc"] for r in res.results]
    eaccs = [r["out_eacc"] for r in res.results]
    return reduce_outputs(laccs, eaccs)


if __name__ == "__main__":
    print("module ok")


# revision 14
# speedup vs baseline: 1.4851x; 1.0737x over previous
"""Trainium2 Bass kernel for nn_DeepModel_70703751626759 (deep-BSDE forward sim).

v3: two software-pipelined sample streams (column halves) so TensorE
never idles (HAM stays warm at 2.4 GHz); dw broadcast moved off the PE
onto DMA (host-replicated rows) so SCA/SCB are bf16 2x-mode DVE ops;
rotated PE tile placement (H1 at p+-1, H2 at p+2, SGR rhs at p+2) to
spread matmuls across the 4x4 sub-array grid; elementwise work split
across ScalarE (state copy + 2 tanh) and VectorE (biased ZU copy,
SCA/SCB, loss accum).

Data-parallel over 8 NeuronCores: 32768 samples -> 4096/core -> 8 column
blocks of 512 samples; block j = p + 4q at partitions [32p, 32p+32),
free cols [512q, 512q+512). Slot content: state = [X(16); Y(16)],
ZU = [u(8); Zv(16); dH(8)]. t folded into layer-1 biases; losses
reduced on-device via accum_out.
"""

import sys
import os
import numpy as np

if "/opt/trn_rl_repo" not in sys.path:
    sys.path.insert(0, "/opt/trn_rl_repo")

N = 16
M = 8
T = 50
DT = 0.01
GAMMA = 0.1
SIGMA = 0.2
TAU = 0.5
H = 10
BATCH = 32768
NCORES = 8
CB = BATCH // NCORES
BK = 512

F32 = np.float32
try:
    import ml_dtypes
    BF16 = ml_dtypes.bfloat16
except ImportError:          # pragma: no cover
    BF16 = np.float32

# bf16 consts image (ckb) column offsets
K_W1 = 0
K_W2 = 32
K_W3 = 64
K_WZY = 96
K_WDXY = 128
K_WDZU = 160
K_WDSA = 192
K_WDSB = 224
K_WE = 256
K_WY1 = 288
K_WY2 = 320
K_WY3 = 352
K_SGR = 384
K_BY3V = 416
K_ONES512 = 448
K_W1X = 1472
K_W1SA = 1504
K_W1ZU = 1536
K_W1SB = 1568
K_SGW = 1600
K_COLS = 1632

# fp32 consts image (ck) column offsets
C_WYP = 0
C_B1 = 32          # 64 cols (t = 0..T-1)
C_B2 = 96
C_B3 = 97
C_BY1 = 98
C_BY2 = 99
C_GB = 100         # 51 cols: t*gamma*dt on X rows (t = 0..T)
C_COLS = 160


def _ct(t):
    w = 1.0 if (t == 0 or t == T - 1) else 2.0
    return 0.5 * DT * w * TAU * TAU


def _b_of(p, q):
    return (p + (1 if q == 0 else 3)) % 4


def _weight_blocks(inp):
    A = np.asarray(inp["A"], F32)
    Bm = np.asarray(inp["Bmat"], F32)
    C = np.asarray(inp["Cmat"], F32)
    D = np.asarray(inp["Dmat"], F32)
    ZW1 = np.asarray(inp["Z_W1"], F32)
    ZW2 = np.asarray(inp["Z_W2"], F32)
    ZW3 = np.asarray(inp["Z_W3"], F32)
    PW1 = np.asarray(inp["phi_W1"], F32)
    PW2 = np.asarray(inp["phi_W2"], F32)
    PW3 = np.asarray(inp["phi_W3"], F32)
    YW1 = np.asarray(inp["Y0_W1"], F32)
    YW2 = np.asarray(inp["Y0_W2"], F32)
    YW3 = np.asarray(inp["Y0_W3"], F32)
    I16 = np.eye(16, dtype=F32)

    def blk():
        return np.zeros((32, 32), F32)

    W1 = blk()
    W1[0:16, 0:10] = ZW1[1:, :]
    W1[0:16, 10:20] = PW1[1:, :]
    W2 = blk()
    W2[0:10, 0:10] = ZW2
    W2[10:20, 10:20] = PW2
    W3 = blk()
    W3[10:20, 0:8] = PW3
    W3[0:10, 8:24] = ZW3
    W3[0:10, 24:32] = ZW3 @ D
    W3[10:20, 24:32] = PW3
    WZY = blk()
    WZY[16:32, 24:32] = Bm
    WDXY = blk()
    WDXY[0:16, 0:16] = DT * A.T
    WDXY[0:16, 16:32] = -DT * I16
    WDXY[16:32, 16:32] = -DT * A
    WDZU = blk()
    WDZU[0:8, 0:16] = DT * Bm.T
    WDZU[8:24, 16:32] = -DT * C
    WDSA = blk()
    WDSA[0:16, 0:16] = C.T
    WDSB = blk()
    WDSB[0:8, 0:16] = D.T
    WDSB[8:24, 16:32] = I16
    WE = blk()
    WE[0:16, 0:16] = -I16
    WE[16:32, 0:16] = I16
    WY1 = blk()
    WY1[0:16, 0:10] = YW1
    WY2 = blk()
    WY2[0:10, 0:10] = YW2
    WY3 = blk()
    WY3[0:10, 16:32] = YW3
    WYP = blk()
    WYP[0:16, 0:16] = I16
    SGR = np.zeros((1, 32), F32)      # K=1 lhsT row: sigma into X rows
    SGR[0, 0:16] = SIGMA
    GDT = np.zeros((1, 32), F32)      # K=1 lhsT row: gamma*dt into X rows
    GDT[0, 0:16] = GAMMA * DT
    BY3V = np.zeros((1, 32), F32)
    BY3V[0, 16:32] = np.asarray(inp["Y0_b3"], F32)
    W1X = W1 + WDXY @ W1
    W1SA = WDSA @ W1
    W1ZU = WDZU @ W1
    W1ZH = W3 @ W1ZU
    W1SB = WDSB @ W1
    SGW = np.zeros((1, 32), F32)
    SGW[0, :] = SIGMA * W1[0:16, :].sum(axis=0)
    return dict(W1=W1, W2=W2, W3=W3, WZY=WZY, WDXY=WDXY, WDZU=WDZU,
                WDSA=WDSA, WDSB=WDSB, WE=WE, WY1=WY1, WY2=WY2, WY3=WY3,
                WYP=WYP, SGR=SGR, GDT=GDT, BY3V=BY3V, W1X=W1X, W1SA=W1SA,
                W1ZU=W1ZU, W1ZH=W1ZH, W1SB=W1SB, SGW=SGW)


def pack_weights_bf16(inp):
    wb = _weight_blocks(inp)
    img = np.zeros((128, K_COLS), F32)
    reps = [(K_W1, "W1"), (K_W2, "W2"), (K_W3, "W3"), (K_WZY, "WZY"),
            (K_WDXY, "WDXY"), (K_WDZU, "WDZU"), (K_WDSA, "WDSA"),
            (K_WDSB, "WDSB"), (K_WE, "WE"), (K_WY1, "WY1"),
            (K_WY2, "WY2"), (K_WY3, "WY3"), (K_W1X, "W1X"),
            (K_W1SA, "W1SA"), (K_W1ZU, "W1ZH"), (K_W1SB, "W1SB")]
    for p in range(4):
        r = 32 * p
        for off, name in reps:
            img[r: r + 32, off: off + 32] = wb[name]
        img[r: r + 1, K_BY3V: K_BY3V + 32] = wb["BY3V"]
        img[r: r + 1, K_SGR: K_SGR + 32] = wb["SGR"]
        img[r: r + 1, K_SGW: K_SGW + 32] = wb["SGW"]
    img[:, K_ONES512: K_ONES512 + 1024] = 1.0
    return img.astype(BF16)


def pack_weights_f32(inp):
    wb = _weight_blocks(inp)
    Zb1 = np.asarray(inp["Z_b1"], F32)
    Zb2 = np.asarray(inp["Z_b2"], F32)
    Zb3 = np.asarray(inp["Z_b3"], F32)
    Pb1 = np.asarray(inp["phi_b1"], F32)
    Pb2 = np.asarray(inp["phi_b2"], F32)
    Pb3 = np.asarray(inp["phi_b3"], F32)
    Yb1 = np.asarray(inp["Y0_b1"], F32)
    Yb2 = np.asarray(inp["Y0_b2"], F32)
    ZW1 = np.asarray(inp["Z_W1"], F32)
    PW1 = np.asarray(inp["phi_W1"], F32)
    D = np.asarray(inp["Dmat"], F32)
    img = np.zeros((128, C_COLS), F32)
    w1sum20 = wb["W1"][0:16, 0:20].sum(axis=0)
    b3v = np.concatenate([Pb3, Zb3, Pb3 + D.T @ Zb3])
    zb3c = wb["W1ZU"][:, 0:20].T @ b3v
    for t in range(T):
        tv = F32(t * DT)
        b = np.concatenate([Zb1 + tv * ZW1[0, :], Pb1 + tv * PW1[0, :]])
        if t >= 1:
            b = b + GAMMA * DT * w1sum20 + zb3c
        for p in range(4):
            img[32 * p: 32 * p + 20, C_B1 + t] = b
    b2 = np.concatenate([Zb2, Pb2])
    b3 = np.concatenate([Pb3, Zb3, Pb3 + D.T @ Zb3])
    for p in range(4):
        r = 32 * p
        img[r: r + 32, C_WYP: C_WYP + 32] = wb["WYP"]
        img[r: r + 20, C_B2] = b2
        img[r: r + 32, C_B3] = b3
        img[r: r + 10, C_BY1] = Yb1
        img[r: r + 10, C_BY2] = Yb2
        for t in range(T + 1):
            img[r: r + 16, C_GB + t] = t * GAMMA * DT
    return img


def pack_x0(X0, core):
    out = np.zeros((128, 1024), F32)
    base = core * CB
    for p in range(4):
        for q in range(2):
            j = p + 4 * q
            out[32 * p: 32 * p + 16, 512 * q: 512 * (q + 1)] = \
                X0[base + BK * j: base + BK * (j + 1), :].T
    return np.ascontiguousarray(out)


def pack_pdw(dw, core):
    """[T*128, 1024] bf16: rows [128t+32p, +32) cols [512q, +512) =
    dw of block j = p+4q at step t, replicated across the 32 rows."""
    base = core * CB
    d = np.empty((T, 4, 2, BK), F32)
    for p in range(4):
        for q in range(2):
            j = p + 4 * q
            d[:, p, q, :] = dw[:, base + BK * j: base + BK * (j + 1), 0]
    # [T, 4, 2, BK] -> [T, 4p, 32rep, 2q*BK]
    out = np.broadcast_to(d[:, :, None, :, :], (T, 4, 32, 2, BK))
    out = out.reshape(T * 128, 1024)
    return np.ascontiguousarray(out.astype(BF16))


# ---------------------------------------------------------------------------
# device program
# ---------------------------------------------------------------------------

_BUILT = {}
STEPOPS = int(os.environ.get("K_STEPOPS", "10"))


def build(t_steps=T):
    if t_steps in _BUILT:
        return _BUILT[t_steps]
    from contextlib import ExitStack
    import concourse.tile as tile
    from concourse import bacc, mybir

    f32 = mybir.dt.float32
    bf16 = mybir.dt.bfloat16
    AF = mybir.ActivationFunctionType
    OP = mybir.AluOpType

    PF = 3                     # pdw prefetch depth (steps ahead)

    nc = bacc.Bacc("TRN2", target_bir_lowering=False, debug=False)
    pdw_d = nc.dram_tensor("pdw", [T * 128, 1024], bf16,
                           kind="ExternalInput").ap()
    x0p_d = nc.dram_tensor("x0p", [128, 1024], f32, kind="ExternalInput").ap()
    ckb_d = nc.dram_tensor("ckb", [128, K_COLS], bf16,
                           kind="ExternalInput").ap()
    ckf_d = nc.dram_tensor("ckf", [128, C_COLS], f32,
                           kind="ExternalInput").ap()
    lacc_d = nc.dram_tensor("out_lacc", [128, 64], f32,
                            kind="ExternalOutput").ap()
    eacc_d = nc.dram_tensor("out_eacc", [128, 8], f32,
                            kind="ExternalOutput").ap()

    def B32(tens, g):
        return tens[32 * g: 32 * g + 32, :]

    with tile.TileContext(nc) as tc, ExitStack() as ctx:
        sb = ctx.enter_context(tc.tile_pool(name="sb", bufs=1))
        dwpool = ctx.enter_context(tc.tile_pool(name="dwp", bufs=PF + 1))
        ps = ctx.enter_context(tc.tile_pool(name="ps", bufs=1, space="PSUM"))

        ckb = sb.tile([128, K_COLS], bf16, tag="ckb")
        ckf = sb.tile([128, C_COLS], f32, tag="ckf")
        nc.sync.dma_start(out=ckb[:, :], in_=ckb_d[:, :])
        nc.sync.dma_start(out=ckf[:, :], in_=ckf_d[:, :])
        X0SB = sb.tile([128, 1024], f32, tag="X0SB")
        nc.sync.dma_start(out=X0SB[:, :], in_=x0p_d[:, :])

        XY = [sb.tile([128, 512], bf16, tag=f"XY{q}", name=f"XY{q}") for q in range(2)]
        H1 = [sb.tile([128, 512], bf16, tag=f"H1{q}", name=f"H1{q}") for q in range(2)]
        H2 = [sb.tile([128, 512], bf16, tag=f"H2{q}", name=f"H2{q}") for q in range(2)]
        ZUm = sb.tile([128, 1024], bf16, tag="ZUm", name="ZUm")
        SCA = [sb.tile([128, 512], bf16, tag=f"SCA{q}", name=f"SCA{q}") for q in range(2)]
        SCB = [sb.tile([128, 512], bf16, tag=f"SCB{q}", name=f"SCB{q}") for q in range(2)]
        SCRm = sb.tile([128, 1024], bf16, tag="SCRm", name="SCRm")
        lacc = sb.tile([128, 64], f32, tag="lacc")
        eacc = sb.tile([128, 8], f32, tag="eacc")
        nc.vector.memset(lacc[:, :], 0.0)
        nc.vector.memset(eacc[:, :], 0.0)

        PST = [ps.tile([128, 512], f32, tag=f"pst{q}", name=f"pst{q}") for q in range(2)]

        def wkb(off, g):
            return ckb[32 * g: 32 * g + 32, off: off + 32]

        def wkf(off, g):
            return ckf[32 * g: 32 * g + 32, off: off + 32]

        def bias(col):
            return ckf[:, col: col + 1]

        def mm(out_t, og, lhsT, rhs, start, stop, rg):
            nc.tensor.matmul(out=B32(out_t, og), lhsT=lhsT, rhs=rhs,
                             start=start, stop=stop,
                             tile_position=(32 * rg, 32 * og),
                             skip_group_check=True)

        # ---- init: PST = WYP @ X0; PST[Y rows] += Y0-MLP(X0) ----
        for q in range(2):
            for p in range(4):
                mm(PST[q], p, wkf(C_WYP, p),
                   X0SB[32 * p: 32 * p + 32, 512 * q: 512 * q + 512],
                   True, False, p)
        ph_i = [ps.tile([128, 512], f32, tag=f"ph1{q}", name=f"ph1{q}") for q in range(2)]
        ph2_i = [ps.tile([128, 512], f32, tag=f"ph2{q}", name=f"ph2i{q}") for q in range(2)]
        for q in range(2):
            nc.scalar.activation(out=XY[q][:, :], in_=PST[q][:, :],
                                 func=AF.Copy)
        for q in range(2):
            for p in range(4):
                mm(ph_i[q], _b_of(p, q), wkb(K_WY1, p), B32(XY[q], p),
                   True, True, p)
        for q in range(2):
            nc.scalar.activation(out=H1[q][:, :], in_=ph_i[q][:, :],
                                 func=AF.Tanh, bias=bias(C_BY1))
        for q in range(2):
            for p in range(4):
                b = _b_of(p, q)
                mm(ph2_i[q], p, wkb(K_WY2, b), B32(H1[q], b), True, True, b)
        for q in range(2):
            nc.scalar.activation(out=H2[q][:, :], in_=ph2_i[q][:, :],
                                 func=AF.Tanh, bias=bias(C_BY2))
        for q in range(2):
            for p in range(4):
                mm(PST[q], p, wkb(K_WY3, p), B32(H2[q], p), False, False, p)
                mm(PST[q], p, ckb[32 * p: 32 * p + 1, K_BY3V: K_BY3V + 32],
                   ckb[32 * p: 32 * p + 1, K_ONES512: K_ONES512 + 512],
                   False, False, p)

        # ---- pdw prefetch for first PF steps ----
        pdw_tiles = {}

        def fetch_pdw(t):
            pt = dwpool.tile([128, 1024], bf16, tag="pdw", name="pdwt")
            nc.sync.dma_start(out=pt[:, :],
                              in_=pdw_d[128 * t: 128 * (t + 1), :])
            pdw_tiles[t] = pt

        for t in range(min(PF, t_steps)):
            fetch_pdw(t)

        # ---- steps ----
        # ph1(t) for t>=1 is accumulated during step t-1 from premultiplied
        # weights: ph1(t) = W1Xt.XY + W1SA.SCA + W1ZU.ZU + W1SB.SCB + SGW.dw
        # (gamma*dt absorbed into the tanh1 bias), so tanh1 never waits on
        # the state copy or a separate W1 matmul.
        ph1 = None
        for t in range(t_steps):
            if t + PF < t_steps:
                fetch_pdw(t + PF)
            pdwt = pdw_tiles.pop(t)
            last = (t == t_steps - 1)

            # state copy (+ t*gamma*dt on X rows) -- off critical path
            nc.scalar.activation(out=XY[0][:, :], in_=PST[0][:, :],
                                 func=AF.Identity, bias=bias(C_GB + t))
            nc.vector.tensor_scalar(XY[1][:, :], PST[1][:, :],
                                    bias(C_GB + t), None, OP.add)
            if t == 0:
                ph1 = [ps.tile([128, 512], f32, tag=f"ph1{q}",
                               name=f"ph1{q}") for q in range(2)]
                for q in range(2):
                    for p in range(4):
                        mm(ph1[q], _b_of(p, q), wkb(K_W1, p), B32(XY[q], p),
                           True, True, p)
            for q in range(2):
                nc.vector.tensor_tensor(out=SCA[q][:, :], in0=XY[q][:, :],
                                        in1=pdwt[:, 512 * q: 512 * q + 512],
                                        op=OP.mult)

            # chain head: tanh1 on the pre-accumulated ph1(t)
            for q in range(2):
                nc.scalar.activation(out=H1[q][:, :], in_=ph1[q][:, :],
                                     func=AF.Tanh, bias=bias(C_B1 + t))
            ph2 = [ps.tile([128, 512], f32, tag=f"ph2{q}", name=f"ph2{q}")
                   for q in range(2)]
            for q in range(2):
                for p in range(4):
                    b = _b_of(p, q)
                    mm(ph2[q], p, wkb(K_W2, b), B32(H1[q], b), True, True, b)
            pzu = [ps.tile([128, 512], f32, tag=f"pzu{q}", name=f"pzu{q}")
                   for q in range(2)]
            for q in range(2):
                for p in range(4):
                    mm(pzu[q], p, wkb(K_WZY, p), B32(XY[q], p),
                       True, False, p)

            for q in range(2):
                nc.scalar.activation(out=H2[q][:, :], in_=ph2[q][:, :],
                                     func=AF.Tanh, bias=bias(C_B2))
            # W3 at the front of the diagonal-tile queue (chain)
            for q in range(2):
                for p in range(4):
                    mm(pzu[q], p, wkb(K_W3, p), B32(H2[q], p), False, True, p)

            # ZU and SCB (fused bias+dw) -- SCB' A first: it is on the chain
            nc.vector.scalar_tensor_tensor(
                out=SCB[0][:, :], in0=pzu[0][:, :], scalar=bias(C_B3),
                in1=pdwt[:, 0:512], op0=OP.add, op1=OP.mult)
            nc.vector.scalar_tensor_tensor(
                out=SCB[1][:, :], in0=pzu[1][:, :], scalar=bias(C_B3),
                in1=pdwt[:, 512:1024], op0=OP.add, op1=OP.mult)
            nc.scalar.activation(out=ZUm[:, 0:512], in_=pzu[0][:, :],
                                 func=AF.Identity, bias=bias(C_B3))
            nc.vector.tensor_scalar(ZUm[:, 512:1024], pzu[1][:, :],
                                    bias(C_B3), None, OP.add)

            # ph1(t+1): W1ZH (on H2) + W1SB (on SCB) are the late writers
            if not last:
                ph1 = [ps.tile([128, 512], f32, tag=f"ph1{q}",
                               name=f"ph1n{q}") for q in range(2)]
                for q in range(2):
                    for p in range(4):
                        b = _b_of(p, q)
                        mm(ph1[q], b, wkb(K_W1ZU, p), B32(H2[q], p),
                           True, False, p)
                        mm(ph1[q], b, wkb(K_W1SB, p), B32(SCB[q], p),
                           False, False, p)
                for q in range(2):
                    for p in range(4):
                        b = _b_of(p, q)
                        mm(ph1[q], b, wkb(K_W1X, p), B32(XY[q], p),
                           False, False, p)
                        mm(ph1[q], b,
                           ckb[32 * p: 32 * p + 1, K_SGW: K_SGW + 32],
                           pdwt[32 * p: 32 * p + 1, 512 * q: 512 * q + 512],
                           False, False, p)
                        mm(ph1[q], b, wkb(K_W1SA, p), B32(SCA[q], p),
                           False, True, p)
            # off-chain diag work after the chain-critical matmuls
            for q in range(2):
                for p in range(4):
                    mm(PST[q], p,
                       ckb[32 * p: 32 * p + 1, K_SGR: K_SGR + 32],
                       pdwt[32 * p: 32 * p + 1, 512 * q: 512 * q + 512],
                       False, False, p)
                    mm(PST[q], p, wkb(K_WDXY, p), B32(XY[q], p),
                       False, False, p)
                    mm(PST[q], p, wkb(K_WDSA, p), B32(SCA[q], p),
                       False, False, p)
                    mm(PST[q], p, wkb(K_WDSB, p), B32(SCB[q], p),
                       False, False, p)
                    mm(PST[q], p, wkb(K_WDZU, p),
                       ZUm[32 * p: 32 * p + 32, 512 * q: 512 * q + 512],
                       False, last and q == 1 and p == 3, p)
            nc.vector.scalar_tensor_tensor(
                out=SCRm[:, :], in0=ZUm[:, :], scalar=float(_ct(t)),
                in1=ZUm[:, :], op0=OP.mult, op1=OP.mult,
                accum_out=lacc[:, t: t + 1])

        # ---- final: loss_bsde = sum((Y - X)^2) ----
        pe = [ps.tile([128, 512], f32, tag=f"ph1{q}", name=f"ph1{q}") for q in range(2)]
        for q in range(2):
            nc.scalar.activation(out=XY[q][:, :], in_=PST[q][:, :],
                                 func=AF.Identity, bias=bias(C_GB + t_steps))
        for q in range(2):
            for p in range(4):
                mm(pe[q], p, wkb(K_WE, p), B32(XY[q], p), True, True, p)
        for q in range(2):
            nc.scalar.activation(out=SCRm[:, 512 * q: 512 * q + 512],
                                 in_=pe[q][:, :], func=AF.Square,
                                 accum_out=eacc[:, q: q + 1])
        nc.sync.dma_start(out=lacc_d[:, :], in_=lacc[:, :])
        nc.sync.dma_start(out=eacc_d[:, :], in_=eacc[:, :])

    nc.compile()
    _BUILT[t_steps] = nc
    return nc


def make_in_maps(inputs):
    ckb = pack_weights_bf16(inputs)
    ckf = pack_weights_f32(inputs)
    X0 = np.asarray(inputs["X0"], F32)
    dw = np.asarray(inputs["dw"], F32)
    in_maps = []
    for k in range(NCORES):
        in_maps.append({
            "pdw": pack_pdw(dw, k),
            "x0p": pack_x0(X0, k),
            "ckb": ckb,
            "ckf": ckf,
        })
    return in_maps


def reduce_outputs(laccs, eaccs, t_steps=T):
    dh_rows = np.zeros(128, bool)
    e_rows = np.zeros(128, bool)
    for p in range(4):
        dh_rows[32 * p + 24: 32 * p + 32] = True
        e_rows[32 * p: 32 * p + 16] = True
    lc = 0.0
    lb = 0.0
    for lacc, eacc in zip(laccs, eaccs):
        lc += float(np.sum(np.asarray(lacc, np.float64)[dh_rows, :t_steps]))
        lb += float(np.sum(np.asarray(eacc, np.float64)[e_rows, 0:2]))
    return np.array([lb / BATCH, lc / BATCH], F32)


def kernel(**inputs):
    from concourse.bass_utils import run_bass_kernel_spmd

    nc = build(T)
    in_maps = make_in_maps(inputs)
    res = run_bass_kernel_spmd(nc, in_maps, core_ids=list(range(NCORES)))
    laccs = [r["out_lacc"] for r in res.results]
    eaccs = [r["out_eacc"] for r in res.results]
    return reduce_outputs(laccs, eaccs)


if __name__ == "__main__":
    print("module ok")
